# revision 1
# baseline (speedup 1.0000x reference)
"""ChebConv (K=3) GNN message-passing kernel for Trainium2, 8 NeuronCores.

Strategy (graph/data parallel, dst-partitioned):
  - Nodes are split into 8 contiguous ranges (12500 per core). Each core owns
    the output rows (and in-edges) of its range.
  - Edges are grouped per core by 128-node destination "window"; each window
    gets a uniform budget of U 128-edge tiles (host-padded).
  - Message gather: per window, an indirect DMA (int32 row indices) pulls
    h[src] rows (bf16, 256B) from an HBM table into SBUF in [128 edges x F]
    tile layout.
  - Aggregation: per 128-edge tile, a one-hot "selection" matrix S
    (host-precomputed, streamed fp8) maps edges to window rows via the
    TensorEngine: psum[n, f] += S[e, n]^T @ msg[e, f], accumulated over the
    window's U tiles.
  - Chebyshev recurrence (Tx1, Tx2) and the final rst = sum_k Txk @ W[k] + b
    run on-chip in fp32; h tables for the next hop are rebuilt per slice and
    exchanged with an AllGather collective across the 8 cores.

Host-side work is limited to graph restructuring (edge partition / sort /
padding, degree counting, one-hot table layout) — all floating-point math on
node features happens on device.
"""

import os
import sys

import numpy as np

sys.path.insert(0, "/opt/trn_rl_repo")

from contextlib import ExitStack

def _ensure_ntff_hook():
    """The agent image's antenv lacks axon_hooks; synthesize the module so
    run_bass_kernel_spmd(trace=True) can reach the NTFF profiler."""
    import types

    if "antenv.axon_hooks" in sys.modules:
        return
    try:
        import trn_agent_boot.trn_boot as tb

        hook = tb._ntff_profile_via_ctypes("/opt/axon/libaxon_pjrt.so")
    except Exception:
        hook = None
    mod = types.ModuleType("antenv.axon_hooks")
    state = {"hook": hook}
    mod.get_axon_ntff_profile_hook = lambda: state["hook"]
    mod.set_axon_ntff_profile_hook = lambda h: state.update(hook=h)
    sys.modules["antenv.axon_hooks"] = mod


_ensure_ntff_hook()

import concourse.bacc as bacc
import concourse.bass as bass
import concourse.tile as tile
from concourse import mybir
from concourse.bass import IndirectOffsetOnAxis
from concourse.bass_utils import run_bass_kernel_spmd
from concourse.masks import make_identity

P = 128
F = 128
K = 3


CH = 32768  # max rows addressable by int16 dma_gather indices


class Cfg:
    def __init__(self, N, E, n_cores, s_dtype=mybir.dt.float8e4):
        self.N = N
        self.E = E
        self.NC = n_cores
        assert N % n_cores == 0
        self.NPC = N // n_cores
        self.W = (self.NPC + P - 1) // P  # windows per core
        self.NCHUNK = (N + CH - 1) // CH
        self.s_dtype = s_dtype


FULL = Cfg(100000, 3200000, 8)
NUM_QUEUES = int(os.environ.get("CHEB_QUEUES", "4"))


def preprocess(cfg, src, dst):
    """Partition edges by (dst core, 128-node window, src chunk); assign
    128-edge tile slots with static per-(window, chunk) tile budgets
    (max over cores, so the SPMD program is identical on every core).

    Returns (budgets, deg, per_core) where budgets is an int array [W, NCHUNK]
    of tile counts, and each per-core entry carries:
      idx16: [128, TOT*8] int16 dma_gather index stream (16-wrapped, x8 repl)
      s:     [128, TOT*F] one-hot S stream (fp8)
    with TOT = budgets.sum() tiles laid out window-major, chunk-sub-blocks.
    """
    N, NC, NPC, W, NK = cfg.N, cfg.NC, cfg.NPC, cfg.W, cfg.NCHUNK
    src = np.asarray(src).astype(np.int64)
    dst = np.asarray(dst).astype(np.int64)
    deg = np.bincount(dst, minlength=N).astype(np.float32)

    cores = []
    cnts = np.zeros((NC, W, NK), dtype=np.int64)
    for c in range(NC):
        base = c * NPC
        sel = (dst >= base) & (dst < base + NPC)
        es = src[sel]
        ed = dst[sel] - base
        wof = ed >> 7
        ch = es // CH
        order = np.lexsort((ch, wof))
        es, ed, wof, ch = es[order], ed[order], wof[order], ch[order]
        np.add.at(cnts[c], (wof, ch), 1)
        cores.append((es, ed, wof, ch))

    budgets = np.ceil(cnts.max(axis=0) / P).astype(np.int64)  # [W, NK]
    tile_off = np.zeros((W, NK), dtype=np.int64)  # global tile index of call
    flat = budgets.reshape(-1)
    off = np.concatenate([[0], np.cumsum(flat)[:-1]]).reshape(W, NK)
    tile_off[:, :] = off
    TOT = int(flat.sum())

    s_np_dtype = mybir.dt.np(cfg.s_dtype)
    one = np.ones(1, dtype=s_np_dtype)[0]
    per_core = []
    for c in range(NC):
        es, ed, wof, ch = cores[c]
        # rank within each (window, chunk) group
        seg_sizes = cnts[c].reshape(-1)
        seg_starts = np.concatenate([[0], np.cumsum(seg_sizes)[:-1]])
        seg_id = wof * NK + ch
        rank = np.arange(len(es)) - seg_starts[seg_id]
        p = (rank % P).astype(np.int64)
        t = rank // P  # tile within the call
        gtile = tile_off[wof, ch] + t  # global tile index

        # int16 index stream, 16-wrapped: slot j -> [j%16, base + j//16]
        idx16 = np.zeros((16, TOT * 8), dtype=np.int16)
        j = rank  # slot within call
        word = tile_off[wof, ch] * 8 + j // 16
        idx16[j % 16, word] = (es - ch * CH).astype(np.int16)
        idx16 = np.tile(idx16, (8, 1))

        dl = ed - (wof << 7)  # 0..127
        s_stream = np.zeros((P, TOT * F), dtype=s_np_dtype)
        s_stream[p, gtile * F + dl] = one
        per_core.append({"idx16": idx16, "s": s_stream})
    return budgets, deg, per_core


def pack_pw(arr_slice, W, fill=0.0):
    """[NPC, ...] node-major -> [128, W * inner] partition/window packed."""
    NPC = arr_slice.shape[0]
    inner = arr_slice.shape[1] if arr_slice.ndim > 1 else 1
    pad_rows = W * P - NPC
    a = arr_slice.reshape(NPC, inner)
    if pad_rows:
        a = np.concatenate(
            [a, np.full((pad_rows, inner), fill, dtype=a.dtype)], axis=0
        )
    return np.ascontiguousarray(
        a.reshape(W, P, inner).transpose(1, 0, 2).reshape(P, W * inner)
    )


def unpack_pw(a, W, NPC, inner):
    """[128, W * inner] -> [NPC, inner]."""
    return np.ascontiguousarray(
        a.reshape(P, W, inner).transpose(1, 0, 2).reshape(W * P, inner)[:NPC]
    )


def build_program(cfg, budgets):
    N, NC, NPC, W, NK = cfg.N, cfg.NC, cfg.NPC, cfg.W, cfg.NCHUNK
    budgets = np.asarray(budgets)
    woff = np.concatenate([[0], np.cumsum(budgets.sum(axis=1))]).astype(int)
    TOT = int(budgets.sum())
    UMAX = int(budgets.sum(axis=1).max())
    dt = mybir.dt
    nc = bacc.Bacc(
        "TRN2",
        target_bir_lowering=False,
        debug=False,
        enable_asserts=False,
        num_devices=NC,
        num_swdge_queues=NUM_QUEUES,
    )

    feat_pw = nc.dram_tensor("feat_pw", [P, W * F], dt.float32, kind="ExternalInput")
    deg_pw = nc.dram_tensor("deg_pw", [P, W], dt.float32, kind="ExternalInput")
    lam_d = nc.dram_tensor("lam_d", [1, 1], dt.float32, kind="ExternalInput")
    wmat_d = nc.dram_tensor("wmat_d", [K * F, F], dt.float32, kind="ExternalInput")
    bvec_d = nc.dram_tensor("bvec_d", [1, F], dt.float32, kind="ExternalInput")
    ident_d = nc.dram_tensor("ident_d", [P, P], dt.float32, kind="ExternalInput")
    idx_d = nc.dram_tensor("idx_d", [P, TOT * 8], dt.int16, kind="ExternalInput")
    s_d = nc.dram_tensor("s_d", [P, TOT * F], cfg.s_dtype, kind="ExternalInput")
    out_pw = nc.dram_tensor("out_pw", [P, W * F], dt.float32, kind="ExternalOutput")

    h_slice = [
        nc.dram_tensor(f"h{i}_slice", [NPC, F], dt.bfloat16, kind="Internal")
        for i in range(2)
    ]
    h_full = [
        nc.dram_tensor(
            f"h{i}_full", [N, F], dt.bfloat16, kind="Internal", addr_space="Shared"
        )
        for i in range(2)
    ]
    groups = [list(range(NC))]

    with TileCtx(nc) as tc, ExitStack() as ctx:
        const = ctx.enter_context(tc.tile_pool(name="const", bufs=1))
        spool = ctx.enter_context(tc.tile_pool(name="spool", bufs=2))
        mpool = ctx.enter_context(tc.tile_pool(name="mpool", bufs=2))
        hpool = ctx.enter_context(tc.tile_pool(name="hpool", bufs=3))
        vpool = ctx.enter_context(tc.tile_pool(name="vpool", bufs=3))
        opool = ctx.enter_context(tc.tile_pool(name="opool", bufs=3))
        apool = ctx.enter_context(tc.tile_pool(name="apool", bufs=2, space="PSUM"))
        ppool = ctx.enter_context(tc.tile_pool(name="ppool", bufs=2, space="PSUM"))
        rpool = ctx.enter_context(tc.tile_pool(name="rpool", bufs=2, space="PSUM"))
        mppool = ctx.enter_context(tc.tile_pool(name="mppool", bufs=1, space="PSUM"))

        # ---- constants / persistent state ----
        tx0 = const.tile([P, W * F], dt.float32)
        nc.sync.dma_start(out=tx0[:], in_=feat_pw.ap())
        tx1 = const.tile([P, W * F], dt.float32)
        wsb = []
        for k in range(K):
            wk = const.tile([P, F], dt.float32, tag=f"wsb{k}")
            nc.sync.dma_start(out=wk[:], in_=wmat_d.ap()[k * F : (k + 1) * F, :])
            wsb.append(wk)
        ident = const.tile([P, P], dt.float32)
        nc.sync.dma_start(out=ident[:], in_=ident_d.ap())
        ones_row = const.tile([1, P], dt.float32)
        nc.vector.memset(ones_row[:], 1.0)
        bvec_sb = const.tile([1, F], dt.float32)
        nc.sync.dma_start(out=bvec_sb[:], in_=bvec_d.ap())

        PREP_MIN = os.environ.get("CHEB_PREP", "") == "min"
        deg_sb = const.tile([P, W], dt.float32)
        nc.sync.dma_start(out=deg_sb[:], in_=deg_pw.ap())
        norm = const.tile([P, W], dt.float32)
        b128 = const.tile([P, F], dt.float32)
        nl = const.tile([P, W], dt.float32)  # norm * 2/lambda
        if PREP_MIN:
            nc.vector.memset(norm[:], 1.0)
            nc.vector.memset(b128[:], 0.0)
            nc.vector.memset(nl[:], 2.0)
        else:
            rec_deg = const.tile([P, W], dt.float32)
            nc.vector.reciprocal(rec_deg[:], deg_sb[:])
            nc.scalar.activation(
                norm[:], rec_deg[:], mybir.ActivationFunctionType.Sqrt
            )

            lam_sb = const.tile([1, 1], dt.float32)
            nc.sync.dma_start(out=lam_sb[:], in_=lam_d.ap())
            lam_half = const.tile([1, 1], dt.float32)
            nc.vector.tensor_scalar(
                lam_half[:], lam_sb[:], 0.5, None, mybir.AluOpType.mult
            )
            lap_sc = const.tile([1, 1], dt.float32)  # 2 / lambda_max
            nc.vector.reciprocal(lap_sc[:], lam_half[:])

            # broadcast 2/lambda to all partitions: ones[1,P]^T @ lap[1,1]
            lap_ps = mppool.tile([P, 1], dt.float32, space="PSUM", tag="mpsum")
            nc.tensor.matmul(
                lap_ps[:], lhsT=ones_row[:], rhs=lap_sc[:], start=True, stop=True
            )
            lap_bc = const.tile([P, 1], dt.float32)
            nc.vector.tensor_copy(out=lap_bc[:], in_=lap_ps[:])
            # bias broadcast to all partitions
            b_ps = mppool.tile([P, F], dt.float32, space="PSUM", tag="mpsum")
            nc.tensor.matmul(
                b_ps[:], lhsT=ones_row[:], rhs=bvec_sb[:], start=True, stop=True
            )
            nc.vector.tensor_copy(out=b128[:], in_=b_ps[:])

            nc.vector.tensor_scalar(
                nl[:], norm[:], lap_bc[:], None, mybir.AluOpType.mult
            )

        def wslice(tile_, w):
            return tile_[:, w * F : (w + 1) * F]

        def valid_rows(w):
            return P if w < W - 1 else NPC - (W - 1) * P

        # ---- phase A: h0 = feat * norm (bf16), slice -> allgather ----
        for w in range(W):
            hb = hpool.tile([P, F], dt.bfloat16, tag="hb")
            nc.vector.tensor_scalar(
                hb[:], wslice(tx0, w), norm[:, w : w + 1], None,
                mybir.AluOpType.mult,
            )
            v = valid_rows(w)
            nc.sync.dma_start(
                out=h_slice[0].ap()[w * P : w * P + v, :], in_=hb[:v, :]
            )
        def allgather(i):
            if os.environ.get("CHEB_NOCC", "0") == "1":
                # debug mode: fake the allgather with a local copy
                for blk in range(0, NPC, P):
                    tmp = hpool.tile([P, F], dt.bfloat16, tag="agtmp")
                    nc.sync.dma_start(
                        out=tmp[:], in_=h_slice[i].ap()[blk : blk + P, :]
                    )
                    nc.sync.dma_start(
                        out=h_full[i].ap()[blk : blk + P, :], in_=tmp[:]
                    )
                return
            nc.gpsimd.collective_compute(
                "AllGather",
                mybir.AluOpType.bypass,
                replica_groups=groups,
                ins=[h_slice[i].ap()],
                outs=[h_full[i].ap()],
            )

        allgather(0)
        STOP = os.environ.get("CHEB_STOP", "")

        def emit_dummy_out():
            for w in range(W):
                ob = opool.tile([P, F], dt.float32, tag="ob")
                nc.vector.tensor_copy(out=ob[:], in_=wslice(tx0, w))
                nc.sync.dma_start(
                    out=out_pw.ap()[:, w * F : (w + 1) * F], in_=ob[:]
                )

        call_counter = [0]

        def spmm(h_full_t, consume):
            skips = os.environ.get("CHEB_SKIP", "").split(",")
            for w in range(W):
                uw = int(budgets[w].sum())
                if uw == 0:
                    continue
                s_sb = spool.tile([P, UMAX * F], cfg.s_dtype, tag="s")
                if "sload" not in skips:
                    nc.sync.dma_start(
                        out=s_sb[:, : uw * F],
                        in_=s_d.ap()[:, woff[w] * F : (woff[w] + uw) * F],
                    )
                else:
                    nc.vector.memset(s_sb[:], 0.0)
                i_sb = spool.tile([P, UMAX * 8], dt.int16, tag="i16")
                if "iload" not in skips:
                    nc.sync.dma_start(
                        out=i_sb[:, : uw * 8],
                        in_=idx_d.ap()[:, woff[w] * 8 : (woff[w] + uw) * 8],
                    )
                else:
                    nc.vector.memset(i_sb[:], 0)
                msg = mpool.tile([P, UMAX, F], dt.bfloat16, tag="msg")
                if "gather" not in skips:
                    # hw limit: one SWDGE gather call handles <=1024 indices
                    # (8 tiles); larger calls overflow the descriptor ring.
                    GMAX = 8
                    bk = 0
                    for k in range(NK):
                        uk = int(budgets[w][k])
                        if uk == 0:
                            continue
                        lo = k * CH
                        hi = min((k + 1) * CH, N)
                        for o in range(0, uk, GMAX):
                            ul = min(GMAX, uk - o)
                            bo = bk + o
                            nc.gpsimd.dma_gather(
                                out_ap=msg[:, bo : bo + ul, :],
                                in_ap=h_full_t.ap()[lo:hi, :],
                                idxs_ap=i_sb[:, bo * 8 : (bo + ul) * 8],
                                num_idxs=ul * P,
                                num_idxs_reg=ul * P,
                                elem_size=F,
                                queue_num=call_counter[0] % NUM_QUEUES,
                            )
                            call_counter[0] += 1
                        bk += uk
                else:
                    nc.vector.memset(msg[:], 0.0)
                agg = apool.tile([P, F], dt.float32, space="PSUM", tag="agg")
                if "matmul" not in skips:
                    for t in range(uw):
                        nc.tensor.matmul(
                            agg[:],
                            lhsT=s_sb[:, t * F : (t + 1) * F],
                            rhs=msg[:, t, :],
                            start=(t == 0),
                            stop=(t == uw - 1),
                        )
                else:
                    nc.vector.memset(agg[:], 0.0)
                consume(w, agg)

        # ---- phase B: Tx1 = spmm(h0) * nl - Tx0 ; h1 = Tx1 * norm ----
        def consume1(w, agg):
            if "consume" in os.environ.get("CHEB_SKIP", "").split(","):
                return
            nc.vector.scalar_tensor_tensor(
                out=wslice(tx1, w),
                in0=agg[:],
                scalar=nl[:, w : w + 1],
                in1=wslice(tx0, w),
                op0=mybir.AluOpType.mult,
                op1=mybir.AluOpType.subtract,
            )
            hb = hpool.tile([P, F], dt.bfloat16, tag="hb")
            nc.vector.tensor_scalar(
                hb[:], wslice(tx1, w), norm[:, w : w + 1], None,
                mybir.AluOpType.mult,
            )
            v = valid_rows(w)
            nc.sync.dma_start(
                out=h_slice[1].ap()[w * P : w * P + v, :], in_=hb[:v, :]
            )

        stopped = False
        if STOP == "A":
            emit_dummy_out()
            stopped = True

        if not stopped:
            spmm(h_full[0], consume1)
            allgather(1)
            if STOP == "B":
                emit_dummy_out()
                stopped = True

        # ---- phase C: Tx2 = 2*(spmm(h1)*nl - Tx1) - Tx0 ;
        #               out = Tx0@W0 + Tx1@W1 + Tx2@W2 + b ----
        def consume2(w, agg):  # noqa: indentation-note below
            _consume2(w, agg)

        def _consume2(w, agg):
            tmp = vpool.tile([P, F], dt.float32, tag="tmp")
            nc.vector.scalar_tensor_tensor(
                out=tmp[:],
                in0=agg[:],
                scalar=nl[:, w : w + 1],
                in1=wslice(tx1, w),
                op0=mybir.AluOpType.mult,
                op1=mybir.AluOpType.subtract,
            )
            tx2 = vpool.tile([P, F], dt.float32, tag="tx2")
            nc.vector.scalar_tensor_tensor(
                out=tx2[:],
                in0=tmp[:],
                scalar=2.0,
                in1=wslice(tx0, w),
                op0=mybir.AluOpType.mult,
                op1=mybir.AluOpType.subtract,
            )
            rst = rpool.tile([P, F], dt.float32, space="PSUM", tag="rst")
            for k, txk in enumerate([wslice(tx0, w), wslice(tx1, w), tx2[:]]):
                tp = ppool.tile([P, F], dt.float32, space="PSUM", tag="tp")
                nc.tensor.transpose(tp[:], txk, ident[:])
                tkT = vpool.tile([P, F], dt.float32, tag="tkT")
                nc.vector.tensor_copy(out=tkT[:], in_=tp[:])
                nc.tensor.matmul(
                    rst[:], lhsT=tkT[:], rhs=wsb[k][:],
                    start=(k == 0), stop=(k == 2),
                )
            ob = opool.tile([P, F], dt.float32, tag="ob")
            nc.vector.tensor_tensor(
                out=ob[:], in0=rst[:], in1=b128[:], op=mybir.AluOpType.add
            )
            nc.sync.dma_start(out=out_pw.ap()[:, w * F : (w + 1) * F], in_=ob[:])

        if not stopped:
            spmm(h_full[1], consume2)

    nc.compile()
    return nc


def TileCtx(nc):
    return tile.TileContext(nc)


def make_in_maps(cfg, deg, pre, feat, W_arr, b, lam):
    NC, NPC, W = cfg.NC, cfg.NPC, cfg.W
    feat = np.asarray(feat, dtype=np.float32)
    wmat = np.asarray(W_arr, dtype=np.float32).reshape(K * F, F)
    bvec = np.asarray(b, dtype=np.float32).reshape(1, F)
    lam2 = np.asarray(lam, dtype=np.float32).reshape(1, 1)
    in_maps = []
    for c in range(NC):
        base = c * NPC
        in_maps.append(
            {
                "feat_pw": pack_pw(feat[base : base + NPC], W),
                "deg_pw": pack_pw(
                    deg[base : base + NPC, None], W, fill=1.0
                ).reshape(P, W),
                "lam_d": lam2,
                "wmat_d": wmat,
                "bvec_d": bvec,
                "ident_d": np.eye(P, dtype=np.float32),
                "idx_d": pre[c]["idx16"],
                "s_d": pre[c]["s"],
            }
        )
    return in_maps


_CACHE = {}


def _get_program(cfg, budgets):
    key = (cfg.N, cfg.E, cfg.NC, budgets.tobytes())
    if key not in _CACHE:
        _CACHE[key] = build_program(cfg, budgets)
    return _CACHE[key]


def kernel(feat, src, dst, W, b, lambda_max):
    cfg = FULL
    budgets, deg, pre = preprocess(cfg, src, dst)
    nc = _get_program(cfg, budgets)
    in_maps = make_in_maps(cfg, deg, pre, feat, W, b, lambda_max)
    res = run_bass_kernel_spmd(
        nc,
        in_maps,
        core_ids=list(range(cfg.NC)),
        trace=os.environ.get("CHEB_TRACE", "0") == "1",
    )
    outs = []
    for c in range(cfg.NC):
        outs.append(unpack_pw(res.results[c]["out_pw"], cfg.W, cfg.NPC, F))
    out = np.concatenate(outs, axis=0).astype(np.float32)
    kernel.last_exec_time_ns = res.exec_time_ns
    return out

